# revision 1
# baseline (speedup 1.0000x reference)
"""Trainium2 Bass kernel for CnnLSTM (conv1x1 -> 2-layer LSTM -> AR decode).

Strategy: pure data parallel over batch (B=256 -> 32 per core x 8 cores).
Device layout is "feature-major": gates live as [128 partitions = G-chunk,
32 free = batch], hidden state as [128 part = h-dim chunk, 2*32], so the
state tile directly provides matmul rhs slices and no transposes are ever
needed.  Matmul operands are fp16 (PE runs 1 cycle/row vs 4 for fp32, and
fp16 weight loads get FWL); all accumulation/state math is fp32.

Host side pre-transposes/permutes weights into lhsT tile layouts, folds
biases, and builds the relayouted input; the device does all O(B*T*G*H)
compute.
"""

import numpy as np

import concourse.bacc as bacc
import concourse.bass as bass
import concourse.mybir as mybir
import concourse.tile as tile
from concourse import bass_utils
from concourse.bass import ds

F16 = mybir.dt.float16
F32 = mybir.dt.float32
AF = mybir.ActivationFunctionType
ALU = mybir.AluOpType
ET = mybir.EngineType

# PSUM gate-slot order: i,i,f,f,o,o,g,g  (PyTorch gate order along G is i,f,g,o)
GPERM = [0, 1, 2, 3, 6, 7, 4, 5]
P = 128
B = 32  # batch per core
NCORES = 8
T_FULL = 2048
WARM_BODY = 64  # warmup steps per For_i iteration
AR_BODY = 16  # AR steps per For_i iteration


# ---------------------------------------------------------------- host prep


def _relay_hh(W):
    # W [1024, 256] -> lhsT tiles [128, 2048] fp16; col block (k*8+s)*128+j
    # holds W.T[k*128+p, GPERM[s]*128+j]
    Wt = W.T.reshape(2, 128, 8, 128)
    Wt = Wt[:, :, GPERM, :]
    return np.ascontiguousarray(
        Wt.transpose(1, 0, 2, 3).reshape(128, 2048)
    ).astype(np.float16)


def prep_shared(inputs):
    f32 = np.float32
    g = lambda n: np.asarray(inputs[n], f32)
    W_ih0, W_hh0 = g("W_ih0"), g("W_hh0")
    W_ih1, W_hh1 = g("W_ih1"), g("W_hh1")
    b0 = g("b_ih0") + g("b_hh0")
    b1 = g("b_ih1") + g("b_hh1")
    conv_w, conv_b = g("conv_w"), g("conv_b")
    lin_w, lin_b = g("lin_w"), g("lin_b")

    # x-projection weights, slot-major: wih0u[p, s*128+j] = W_ih0.T[p, GPERM[s]*128+j]
    # (base_partition-64 row packing crashes the device, so keep all MMs at
    # partitions 0:64)
    Wt0 = W_ih0.T.reshape(64, 8, 128)[:, GPERM, :]
    wih0u = np.ascontiguousarray(Wt0.reshape(64, 1024)).astype(np.float16)

    def bias_sb(b):
        br = b.reshape(8, 128)[GPERM]  # [s, p]
        return np.ascontiguousarray(
            np.broadcast_to(br.T[:, :, None], (128, 8, 32)).reshape(128, 256)
        ).astype(f32)

    cw2 = np.tile(conv_w, 2)
    cb2 = np.tile(conv_b, 2)
    return {
        "whh0": _relay_hh(W_hh0),
        "wih1": _relay_hh(W_ih1),
        "whh1": _relay_hh(W_hh1),
        "wih0u": wih0u,
        "b0sb": bias_sb(b0),
        "b1sb": bias_sb(b1),
        # interleaved [cw[c], cb[c]] pairs, replicated across partitions:
        # cwcb[p, 2c] = conv_w[c], cwcb[p, 2c+1] = conv_b[c]
        "cwcb": np.ascontiguousarray(
            np.broadcast_to(
                np.stack([conv_w, conv_b], axis=1).reshape(1, 128), (128, 128)
            )
        ).astype(f32),
        "linwT": np.ascontiguousarray(lin_w[0].reshape(2, 128).T).astype(np.float16),
        "cwrow": cw2[None, :].astype(np.float16),
        "cb2col": (lin_b[0] * cw2 + cb2).astype(f32)[:, None],
        "linbcol": np.full((32, 1), lin_b[0], f32),
    }


def prep_core_input(input_full, core):
    # inpT[p, r*32+b] = input[32*core+b, 64*r + p%64], duplicated rows 64:128
    x = np.asarray(input_full, np.float32)[32 * core : 32 * core + 32]
    x = x.reshape(32, 32, 64)  # [b, r, k]
    one = x.transpose(2, 1, 0).reshape(64, 1024)  # [k, (r b)]
    return np.ascontiguousarray(np.concatenate([one, one], axis=0))


# ---------------------------------------------------------------- device IR


def build_program(T=T_FULL, NP=512, mode="full"):
    assert T % WARM_BODY == 0 and T <= T_FULL
    assert 2 <= NP <= 512
    nc = bacc.Bacc("TRN2", debug=False, enable_asserts=False, num_devices=NCORES)

    def din(name, shape, dt):
        return nc.dram_tensor(name, list(shape), dt, kind="ExternalInput").ap()

    t = {
        "whh0": din("whh0", (128, 2048), F16),
        "wih1": din("wih1", (128, 2048), F16),
        "whh1": din("whh1", (128, 2048), F16),
        "wih0u": din("wih0u", (64, 1024), F16),
        "inpT": din("inpT", (128, 1024), F32),
        "b0sb": din("b0sb", (128, 256), F32),
        "b1sb": din("b1sb", (128, 256), F32),
        "cwcb": din("cwcb", (128, 128), F32),
        "linwT": din("linwT", (128, 2), F16),
        "cwrow": din("cwrow", (1, 128), F16),
        "cb2col": din("cb2col", (128, 1), F32),
        "linbcol": din("linbcol", (32, 1), F32),
    }
    if mode in ("warm", "step1", "step8", "loop8", "mm_hh", "mm_x", "mm_tt", "cellonly"):
        out_ap = nc.dram_tensor("out", [128, 128], F32, kind="ExternalOutput").ap()
    else:
        out_ap = nc.dram_tensor("out", [32, NP], F32, kind="ExternalOutput").ap()

    with tile.TileContext(nc) as tc:
        _emit(tc, nc, t, out_ap, T, NP, mode)
    nc.compile()
    return nc


def _emit(tc, nc, t, out_ap, T, NP, mode="full"):
    import contextlib

    with contextlib.ExitStack() as ctx:
        const = ctx.enter_context(tc.tile_pool(name="const", bufs=1))

        def load(name, shape, dt):
            tl = const.tile(list(shape), dt, tag=name)
            nc.sync.dma_start(tl[:], t[name])
            return tl

        whh0 = load("whh0", (128, 2048), F16)
        wih1 = load("wih1", (128, 2048), F16)
        whh1 = load("whh1", (128, 2048), F16)
        wih0u = load("wih0u", (64, 1024), F16)
        inpT = load("inpT", (128, 1024), F32)
        b0sb = load("b0sb", (128, 256), F32)
        b1sb = load("b1sb", (128, 256), F32)
        cwcb = load("cwcb", (128, 128), F32)
        linwT = load("linwT", (128, 2), F16)
        cwrow = load("cwrow", (1, 128), F16)
        cb2col = load("cb2col", (128, 1), F32)
        linbcol = load("linbcol", (32, 1), F32)

        # persistent state
        h0 = const.tile([128, 64], F16, tag="h0")
        c0 = const.tile([128, 64], F32, tag="c0")
        h1 = const.tile([128, 64], F16, tag="h1")
        c1 = const.tile([128, 64], F32, tag="c1")
        for st in (h0, c0, h1, c1):
            nc.vector.memset(st[:], 0.0)
        p16 = const.tile([1, 32], F16, tag="p16")
        preds = const.tile([32, NP], F32, tag="preds")

        gpool = ctx.enter_context(tc.tile_pool(name="gates", bufs=2, space="PSUM"))
        spool = ctx.enter_context(tc.tile_pool(name="sg", bufs=2))
        tpool = ctx.enter_context(tc.tile_pool(name="tmp", bufs=2))
        xpool = ctx.enter_context(tc.tile_pool(name="xt", bufs=2))
        appool = ctx.enter_context(tc.tile_pool(name="arp", bufs=1, space="PSUM"))

        def cell(g, tagp, h_st, c_st):
            sg = spool.tile([128, 192], F32, tag=tagp + "s")
            nc.scalar.activation(sg[:], g[:, 0:192], AF.Sigmoid)
            gt = tpool.tile([128, 64], F32, tag=tagp + "g")
            nc.scalar.activation(gt[:], g[:, 192:256], AF.Tanh)
            m1 = tpool.tile([128, 64], F32, tag=tagp + "m1")
            nc.vector.tensor_mul(m1[:], sg[:, 64:128], c_st[:])
            m2 = tpool.tile([128, 64], F32, tag=tagp + "m2")
            nc.vector.tensor_mul(m2[:], sg[:, 0:64], gt[:])
            nc.vector.tensor_add(c_st[:], m1[:], m2[:])
            tcc = tpool.tile([128, 64], F32, tag=tagp + "t")
            nc.scalar.activation(tcc[:], c_st[:], AF.Tanh)
            nc.vector.tensor_mul(h_st[:], sg[:, 128:192], tcc[:])

        def hh16(g, w, rhs_tile, first):
            for s in range(8):
                for k in range(2):
                    nc.tensor.matmul(
                        g[:, s * 32 : (s + 1) * 32],
                        lhsT=w[:, (k * 8 + s) * 128 : (k * 8 + s + 1) * 128],
                        rhs=rhs_tile[:, k * 32 : (k + 1) * 32],
                        start=(first and s == 0 and k == 0),
                        stop=False,
                    )

        def layer0(xt):
            g = gpool.tile([128, 256], F32, tag="g0")
            hh16(g, whh0, h0, True)
            for s in range(8):
                nc.tensor.matmul(
                    g[:, s * 32 : (s + 1) * 32],
                    lhsT=wih0u[:, s * 128 : (s + 1) * 128],
                    rhs=xt[0:64, :],
                    start=False,
                    stop=(s == 7),
                )
            nc.vector.tensor_tensor(g[:], g[:], b0sb[:], op=ALU.add)
            cell(g, "l0", h0, c0)

        def layer1():
            g = gpool.tile([128, 256], F32, tag="g1")
            hh16(g, whh1, h1, True)
            for s in range(8):
                for k in range(2):
                    nc.tensor.matmul(
                        g[:, s * 32 : (s + 1) * 32],
                        lhsT=wih1[:, (k * 8 + s) * 128 : (k * 8 + s + 1) * 128],
                        rhs=h0[:, k * 32 : (k + 1) * 32],
                        start=False,
                        stop=(s == 7 and k == 1),
                    )
            nc.vector.tensor_tensor(g[:], g[:], b1sb[:], op=ALU.add)
            cell(g, "l1", h1, c1)

        def warm_step(cur, u):
            # cur: [128, 2] snapshot of (conv_w[c], conv_b[c]) for this chunk
            xt = xpool.tile([128, 32], F16, tag="xt")
            r = u % 32
            nc.scalar.activation(
                xt[:],
                inpT[:, r * 32 : (r + 1) * 32],
                AF.Relu,
                bias=cur[:, 1:2],
                scale=cur[:, 0:1],
            )
            layer0(xt)
            layer1()

        preds_ps = appool.tile([32, NP], F32, tag="predps")

        def pred_head(col):
            for k in range(2):
                nc.tensor.matmul(
                    preds_ps[:, ds(col, 1)],
                    lhsT=h1[:, k * 32 : (k + 1) * 32],
                    rhs=linwT[:, k : k + 1],
                    start=(k == 0),
                    stop=(k == 1),
                )
            prow = appool.tile([1, 32], F32, tag="prow")
            for k in range(2):
                nc.tensor.matmul(
                    prow[:],
                    lhsT=linwT[:, k : k + 1],
                    rhs=h1[:, k * 32 : (k + 1) * 32],
                    start=(k == 0),
                    stop=(k == 1),
                )
            nc.vector.tensor_copy(p16[:], prow[:])

        def ar_step(col):
            cps = appool.tile([128, 32], F32, tag="cps")
            nc.tensor.matmul(cps[:], lhsT=cwrow[:], rhs=p16[:], start=True, stop=True)
            xt = xpool.tile([128, 32], F16, tag="xt")
            nc.scalar.activation(xt[:], cps[:], AF.Relu, bias=cb2col[:])
            layer0(xt)
            layer1()
            pred_head(col)

        hints = (ET.PE, ET.DVE, ET.Activation)

        if mode in ("mm_hh", "mm_x", "mm_tt", "cellonly"):
            xt = xpool.tile([128, 32], F16, tag="xt")
            nc.scalar.activation(xt[:], inpT[:, 0:32], AF.Relu,
                                 bias=cwcb[:, 1:2], scale=cwcb[:, 0:1])
            g = gpool.tile([128, 256], F32, tag="g0")
            if mode == "mm_hh":
                for s in range(8):
                    for k in range(2):
                        nc.tensor.matmul(
                            g[:, s * 32 : (s + 1) * 32],
                            lhsT=whh0[:, (k * 8 + s) * 128 : (k * 8 + s + 1) * 128],
                            rhs=h0[:, k * 32 : (k + 1) * 32],
                            start=(s == 0 and k == 0),
                            stop=(s == 7 and k == 1),
                        )
            elif mode == "mm_x":
                for s in range(8):
                    nc.tensor.matmul(
                        g[:, s * 32 : (s + 1) * 32],
                        lhsT=wih0u[:, s * 128 : (s + 1) * 128],
                        rhs=xt[0:64, :],
                        start=(s == 0),
                        stop=(s == 7),
                    )
            elif mode == "mm_tt":
                nc.vector.memset(g[:], 0.125)
                nc.vector.tensor_tensor(g[:], g[:], b0sb[:], op=ALU.add)
            elif mode == "cellonly":
                nc.vector.memset(g[:], 0.25)
                cell(g, "l0", h0, c0)
            dbg = const.tile([128, 128], F32, tag="dbg")
            if mode == "cellonly":
                nc.vector.tensor_copy(dbg[:, 0:64], h0[:])
                nc.vector.tensor_copy(dbg[:, 64:128], c0[:])
            else:
                nc.vector.tensor_copy(dbg[:], g[:, 0:128])
            nc.sync.dma_start(out_ap, dbg[:])
            return

        if mode in ("step1", "step8", "loop8"):
            nsteps = {"step1": 1, "step8": 8}.get(mode)
            if mode == "loop8":
                with tc.For_i(0, 2, 1, hint_engines=hints) as iv:
                    for u in range(4):
                        warm_step(cwcb[:, 0:2], u)
            else:
                for u in range(nsteps):
                    warm_step(cwcb[:, 0:2], u)
            dbg = const.tile([128, 128], F32, tag="dbg")
            nc.vector.tensor_copy(dbg[:, 0:64], h1[:])
            nc.vector.tensor_copy(dbg[:, 64:128], c1[:])
            nc.sync.dma_start(out_ap, dbg[:])
            return

        if mode == "ar":
            wtrip = 0
        else:
            wtrip = T // WARM_BODY
        if wtrip > 0:
          with tc.For_i(0, wtrip, 1, hint_engines=hints) as iv:
            # ACT scale/bias operands do not support register offsets on HW,
            # so snapshot this body's two (cw, cb) pairs into static tiles
            # via DMA (register offsets on DMA are fine).
            curs = []
            for half in range(2):
                cur = xpool.tile([128, 2], F32, tag="cwcur")
                nc.sync.dma_start(cur[:], cwcb[:, ds(iv * 4 + half * 2, 2)])
                curs.append(cur)
            for u in range(WARM_BODY):
                warm_step(curs[u // 32], u)

        if mode == "warm":
            dbg = const.tile([128, 128], F32, tag="dbg")
            nc.vector.tensor_copy(dbg[:, 0:64], h1[:])
            nc.vector.tensor_copy(dbg[:, 64:128], c1[:])
            nc.sync.dma_start(out_ap, dbg[:])
            return

        pred_head(0)

        nar = NP - 1
        artrip = nar // AR_BODY
        rem = nar - artrip * AR_BODY
        if artrip > 0:
            with tc.For_i(0, artrip, 1, hint_engines=hints) as av:
                for u in range(AR_BODY):
                    ar_step(av * AR_BODY + (u + 1))
        for u in range(rem):
            ar_step(artrip * AR_BODY + u + 1)

        nc.vector.tensor_scalar_add(preds[:], preds_ps[:], linbcol[:])
        nc.sync.dma_start(out_ap, preds[:])


# ---------------------------------------------------------------- entry


def make_in_maps(inputs, ncores=NCORES):
    shared = prep_shared(inputs)
    return [
        dict(shared, inpT=prep_core_input(inputs["input"], c)) for c in range(ncores)
    ]


_PROG_CACHE = {}


def kernel(**inputs):
    inp = np.asarray(inputs["input"], np.float32)
    assert inp.shape == (256, 2048), inp.shape
    NP = int(inputs["num_predictions"])
    if NP not in _PROG_CACHE:
        _PROG_CACHE[NP] = build_program(T_FULL, NP)
    nc = _PROG_CACHE[NP]
    in_maps = make_in_maps(inputs)
    res = bass_utils.run_bass_kernel_spmd(nc, in_maps, core_ids=list(range(NCORES)))
    return np.concatenate([r["out"] for r in res.results], axis=0)


if __name__ == "__main__":
    import reference

    inputs = {k: np.asarray(v) for k, v in reference.setup_inputs().items()}
    out = kernel(**inputs)
    exp = np.asarray(reference.reference(**reference.setup_inputs()))
    err = np.abs(out - exp).max()
    print("absmax err", err, "rel", err / np.abs(exp).max())



# revision 6
# speedup vs baseline: 1.3788x; 1.3788x over previous
"""Trainium2 Bass kernel for CnnLSTM (conv1x1 -> 2-layer LSTM -> AR decode).

Strategy: pure data parallel over batch (B=256 -> 32 per core x 8 cores).
Feature-major layout: gates live as [128 partitions = G-chunk, 32 free =
batch], hidden state as [128 part = h-dim chunk, 2*32].  Matmul operands are
fp16 (FWL weight loads, ~37ns per LDW+MM pair at N=32); accumulation/state
math is fp32.

Warmup pipelining: layer1 runs TWO steps behind layer0, so each tick carries
two independent dependency chains (cell0 of step t, cell1 of step t-2) and
the per-tick critical path is a single cell chain while PE/ACT/DVE overlap
across the two cells.  Biases are folded into the PSUM accumulation via a
one-hot matmul (lhsT = biasT [8,128], rhs = one-hot [8,256]) instead of a
vector add on the critical path.

Host side pre-transposes/permutes weights into lhsT tile layouts, folds
biases, and builds the relayouted input; the device does all O(B*T*G*H)
compute.
"""

import numpy as np

import concourse.bacc as bacc
import concourse.bass as bass
import concourse.mybir as mybir
import concourse.tile as tile
from concourse import bass_utils
from concourse.bass import ds

F16 = mybir.dt.float16
F32 = mybir.dt.float32
AF = mybir.ActivationFunctionType
ALU = mybir.AluOpType
ET = mybir.EngineType

# PSUM gate-slot order: i,i,f,f,o,o,g,g  (PyTorch gate order along G is i,f,g,o)
GPERM = [0, 1, 2, 3, 6, 7, 4, 5]
P = 128
B = 32  # batch per core
NCORES = 8
T_FULL = 2048
WARM_BODY = 64  # steady warmup ticks per For_i iteration
AR_BODY = 16  # AR steps per For_i iteration


# ---------------------------------------------------------------- host prep


def _relay_hh(W):
    # W [1024, 256] -> lhsT tiles [128, 2048] fp16; col block (k*8+s)*128+j
    # holds W.T[k*128+p, GPERM[s]*128+j]
    Wt = W.T.reshape(2, 128, 8, 128)
    Wt = Wt[:, :, GPERM, :]
    return np.ascontiguousarray(
        Wt.transpose(1, 0, 2, 3).reshape(128, 2048)
    ).astype(np.float16)


def prep_shared(inputs):
    f32 = np.float32
    g = lambda n: np.asarray(inputs[n], f32)
    W_ih0, W_hh0 = g("W_ih0"), g("W_hh0")
    W_ih1, W_hh1 = g("W_ih1"), g("W_hh1")
    b0 = g("b_ih0") + g("b_hh0")
    b1 = g("b_ih1") + g("b_hh1")
    conv_w, conv_b = g("conv_w"), g("conv_b")
    lin_w, lin_b = g("lin_w"), g("lin_b")

    # x-projection weights, slot-major: wih0u[p, s*128+j] = W_ih0.T[p, GPERM[s]*128+j]
    # (base_partition-64 row packing crashes the device, so keep all MMs at
    # partitions 0:64)
    Wt0 = W_ih0.T.reshape(64, 8, 128)[:, GPERM, :]
    wih0u = np.ascontiguousarray(Wt0.reshape(64, 1024)).astype(np.float16)

    def bias_lhsT(b):
        # biasT[s, p] = b[GPERM[s]*128 + p]  -> [8, 128] fp16
        br = b.reshape(8, 128)[GPERM]
        return np.ascontiguousarray(br).astype(np.float16)

    # one-hot rhs [8, 256]: onehot[s, g*32 + b] = (g == s)
    onehot = np.zeros((8, 256), np.float16)
    for s in range(8):
        onehot[s, s * 32 : (s + 1) * 32] = 1.0

    cw2 = np.tile(conv_w, 2)
    cb2 = np.tile(conv_b, 2)
    return {
        "whh0": _relay_hh(W_hh0),
        "wih1": _relay_hh(W_ih1),
        "whh1": _relay_hh(W_hh1),
        "wih0u": wih0u,
        "b0T": bias_lhsT(b0),
        "b1T": bias_lhsT(b1),
        "onehot": onehot,
        # interleaved [cw[c], cb[c]] pairs, replicated across partitions:
        # cwcb[p, 2c] = conv_w[c], cwcb[p, 2c+1] = conv_b[c]
        "cwcb": np.ascontiguousarray(
            np.broadcast_to(
                np.stack([conv_w, conv_b], axis=1).reshape(1, 128), (128, 128)
            )
        ).astype(f32),
        "linwT": np.ascontiguousarray(lin_w[0].reshape(2, 128).T).astype(np.float16),
        "cwrow": cw2[None, :].astype(np.float16),
        "cb2col": (lin_b[0] * cw2 + cb2).astype(f32)[:, None],
        "linbcol": np.full((32, 1), lin_b[0], f32),
    }


def prep_core_input(input_full, core):
    # inpT[p, r*32+b] = input[32*core+b, 64*r + p%64], duplicated rows 64:128
    x = np.asarray(input_full, np.float32)[32 * core : 32 * core + 32]
    x = x.reshape(32, 32, 64)  # [b, r, k]
    one = x.transpose(2, 1, 0).reshape(64, 1024)  # [k, (r b)]
    return np.ascontiguousarray(np.concatenate([one, one], axis=0))


# ---------------------------------------------------------------- device IR


def build_program(T=T_FULL, NP=512, mode="full"):
    assert T >= 8 and T <= T_FULL
    assert 2 <= NP <= 512
    nc = bacc.Bacc("TRN2", debug=False, enable_asserts=False, num_devices=NCORES)

    def din(name, shape, dt):
        return nc.dram_tensor(name, list(shape), dt, kind="ExternalInput").ap()

    t = {
        "whh0": din("whh0", (128, 2048), F16),
        "wih1": din("wih1", (128, 2048), F16),
        "whh1": din("whh1", (128, 2048), F16),
        "wih0u": din("wih0u", (64, 1024), F16),
        "inpT": din("inpT", (128, 1024), F32),
        "b0T": din("b0T", (8, 128), F16),
        "b1T": din("b1T", (8, 128), F16),
        "onehot": din("onehot", (8, 256), F16),
        "cwcb": din("cwcb", (128, 128), F32),
        "linwT": din("linwT", (128, 2), F16),
        "cwrow": din("cwrow", (1, 128), F16),
        "cb2col": din("cb2col", (128, 1), F32),
        "linbcol": din("linbcol", (32, 1), F32),
    }
    if mode in ("warm",):
        out_ap = nc.dram_tensor("out", [128, 128], F32, kind="ExternalOutput").ap()
    else:
        out_ap = nc.dram_tensor("out", [32, NP], F32, kind="ExternalOutput").ap()

    with tile.TileContext(nc) as tc:
        _emit(tc, nc, t, out_ap, T, NP, mode)
    nc.compile()
    return nc


def _emit(tc, nc, t, out_ap, T, NP, mode="full"):
    import contextlib

    with contextlib.ExitStack() as ctx:
        const = ctx.enter_context(tc.tile_pool(name="const", bufs=1))

        def load(name, shape, dt):
            tl = const.tile(list(shape), dt, tag=name)
            nc.sync.dma_start(tl[:], t[name])
            return tl

        whh0 = load("whh0", (128, 2048), F16)
        wih1 = load("wih1", (128, 2048), F16)
        whh1 = load("whh1", (128, 2048), F16)
        wih0u = load("wih0u", (64, 1024), F16)
        inpT = load("inpT", (128, 1024), F32)
        b0T = load("b0T", (8, 128), F16)
        b1T = load("b1T", (8, 128), F16)
        onehot = load("onehot", (8, 256), F16)
        cwcb = load("cwcb", (128, 128), F32)
        linwT = load("linwT", (128, 2), F16)
        cwrow = load("cwrow", (1, 128), F16)
        cb2col = load("cb2col", (128, 1), F32)
        linbcol = load("linbcol", (32, 1), F32)

        # persistent state.  h0 is a ring of 4 (cell1 lags cell0 by 2 steps);
        # h1/c0/c1 are single tiles.
        NH0 = 4
        h0r = []
        for i in range(NH0):
            h0i = const.tile([128, 64], F16, tag=f"h0r{i}", name=f"h0r{i}")
            h0r.append(h0i)
        c0 = const.tile([128, 64], F32, tag="c0")
        h1 = const.tile([128, 64], F16, tag="h1")
        c1 = const.tile([128, 64], F32, tag="c1")
        for st in (*h0r, c0, h1, c1):
            nc.vector.memset(st[:], 0.0)
        p16 = const.tile([1, 32], F16, tag="p16")
        preds = const.tile([32, NP], F32, tag="preds")

        g0pool = ctx.enter_context(tc.tile_pool(name="g0", bufs=2, space="PSUM"))
        g1pool = ctx.enter_context(tc.tile_pool(name="g1", bufs=2, space="PSUM"))
        spool = ctx.enter_context(tc.tile_pool(name="sg", bufs=3))
        tpool = ctx.enter_context(tc.tile_pool(name="tmp", bufs=3))
        xpool = ctx.enter_context(tc.tile_pool(name="xt", bufs=3))
        appool = ctx.enter_context(tc.tile_pool(name="arp", bufs=1, space="PSUM"))

        def bias_mm(g, bT, first):
            # g[:, s*32:(s+1)*32] += bias[GPERM[s]*128 + p]
            nc.tensor.matmul(
                g[:, 0:256], lhsT=bT[:], rhs=onehot[:], start=first, stop=False
            )

        def hh16(g, w, rhs_lo, rhs_hi, stop=False):
            for s in range(8):
                for k in range(2):
                    nc.tensor.matmul(
                        g[:, s * 32 : (s + 1) * 32],
                        lhsT=w[:, (k * 8 + s) * 128 : (k * 8 + s + 1) * 128],
                        rhs=rhs_lo if k == 0 else rhs_hi,
                        start=False,
                        stop=stop and (s == 7 and k == 1),
                    )

        def cell_math(g, tagp, h_dst, c_st):
            # g: [128, 256] PSUM  (slots i,i,f,f,o,o,g,g)
            sg = spool.tile([128, 192], F32, tag=tagp + "s")
            nc.scalar.activation(sg[:], g[:, 0:192], AF.Sigmoid)
            gt = tpool.tile([128, 64], F32, tag=tagp + "g")
            nc.scalar.activation(gt[:], g[:, 192:256], AF.Tanh)
            m1 = tpool.tile([128, 64], F32, tag=tagp + "m1")
            nc.vector.tensor_mul(m1[:], sg[:, 64:128], c_st[:])
            m2 = tpool.tile([128, 64], F32, tag=tagp + "m2")
            nc.vector.tensor_mul(m2[:], sg[:, 0:64], gt[:])
            nc.vector.tensor_add(c_st[:], m1[:], m2[:])
            tcc = tpool.tile([128, 64], F32, tag=tagp + "t")
            nc.scalar.activation(tcc[:], c_st[:], AF.Tanh)
            nc.vector.tensor_mul(h_dst[:], sg[:, 128:192], tcc[:])

        def make_xt(cur_or_static, r):
            # xt = relu(cw * u_r + cb); cur_or_static is a [128,2] snapshot
            # tile (loop body) or a static cwcb column pair (unrolled code).
            xt = xpool.tile([128, 32], F16, tag="xt")
            nc.scalar.activation(
                xt[:],
                inpT[:, r * 32 : (r + 1) * 32],
                AF.Relu,
                bias=cur_or_static[:, 1:2],
                scale=cur_or_static[:, 0:1],
            )
            return xt

        def cell0(xt, h0_in, h0_out):
            # gates0 = b0 + W_ih0 @ x + W_hh0 @ h0_in
            g = g0pool.tile([128, 256], F32, tag="g0")
            bias_mm(g, b0T, True)
            for s in range(8):
                nc.tensor.matmul(
                    g[:, s * 32 : (s + 1) * 32],
                    lhsT=wih0u[:, s * 128 : (s + 1) * 128],
                    rhs=xt[0:64, :],
                    start=False,
                    stop=False,
                )
            hh16(g, whh0, h0_in[:, 0:32], h0_in[:, 32:64], stop=True)
            cell_math(g, "l0", h0_out, c0)

        def cell1(h0_in):
            # gates1 = b1 + W_ih1 @ h0_in + W_hh1 @ h1
            g = g1pool.tile([128, 256], F32, tag="g1")
            bias_mm(g, b1T, True)
            hh16(g, wih1, h0_in[:, 0:32], h0_in[:, 32:64])
            hh16(g, whh1, h1[:, 0:32], h1[:, 32:64], stop=True)
            cell_math(g, "l1", h1, c1)

        def warm_tick(s, cur0):
            # steady tick: cell0 of step s (s>=3), cell1 of step s-2 (>=1).
            # x for (1-based) step s is reference step t = s-1.
            xt = make_xt(cur0, (s - 1) % 32)
            cell1(h0r[(s - 2) % NH0])
            cell0(xt, h0r[(s - 1) % NH0], h0r[s % NH0])

        preds_ps = appool.tile([32, NP], F32, tag="predps")

        def pred_head(col):
            for k in range(2):
                nc.tensor.matmul(
                    preds_ps[:, ds(col, 1)],
                    lhsT=h1[:, k * 32 : (k + 1) * 32],
                    rhs=linwT[:, k : k + 1],
                    start=(k == 0),
                    stop=(k == 1),
                )
            prow = appool.tile([1, 32], F32, tag="prow")
            for k in range(2):
                nc.tensor.matmul(
                    prow[:],
                    lhsT=linwT[:, k : k + 1],
                    rhs=h1[:, k * 32 : (k + 1) * 32],
                    start=(k == 0),
                    stop=(k == 1),
                )
            nc.vector.tensor_copy(p16[:], prow[:])

        def ar_step(col):
            # conv: x = relu(cw * p + (lin_b*cw + cb)) via K=1 outer-product MM
            cps = appool.tile([128, 32], F32, tag="cps")
            nc.tensor.matmul(cps[:], lhsT=cwrow[:], rhs=p16[:], start=True, stop=True)
            xt = xpool.tile([128, 32], F16, tag="xt")
            nc.scalar.activation(xt[:], cps[:], AF.Relu, bias=cb2col[:])
            cell0(xt, h0r[0], h0r[0])
            cell1(h0r[0])
            pred_head(col)

        hints = (ET.PE, ET.DVE, ET.Activation)

        # ---------------- warmup scan (ticks with layer1 lagging by 2) ----
        if mode != "ar":
            # prologue: cell0 steps 1 and 2 (no cell1 yet)
            for s in (1, 2):
                p_ = (s - 1) // 32
                xt = make_xt(cwcb[:, 2 * p_ : 2 * p_ + 2], (s - 1) % 32)
                cell0(xt, h0r[(s - 1) % NH0], h0r[s % NH0])

            # steady ticks: s = 3 .. T  (cell0 step s, cell1 step s-2)
            nsteady = T - 2
            ntrip = max(0, (nsteady - 2) // WARM_BODY)
            loop_end = 3 + ntrip * WARM_BODY  # first static-tail step
            if ntrip > 0:
                with tc.For_i(0, ntrip, 1, hint_engines=hints) as iv:
                    # ACT scale/bias operands do not support register offsets
                    # on HW, so snapshot this body's three (cw, cb) pairs into
                    # static tiles via DMA (register offsets on DMA are fine).
                    curs = []
                    for third in range(3):
                        cur = xpool.tile([128, 2], F32, tag=f"cwcur{third}")
                        nc.sync.dma_start(cur[:], cwcb[:, ds(iv * 4 + third * 2, 2)])
                        curs.append(cur)
                    for j in range(WARM_BODY):
                        # actual step s = 3 + iv*WARM_BODY + j; x index t =
                        # s-1 = 2 + iv*64 + j; t//32 = 2*iv + (2+j)//32.
                        warm_tick(3 + j, curs[(2 + j) // 32])
            # static tail: steps loop_end .. T
            for s in range(loop_end, T + 1):
                p_ = (s - 1) // 32
                warm_tick(s, cwcb[:, 2 * p_ : 2 * p_ + 2])
            # epilogue: cell1 steps T-1, T
            cell1(h0r[(T - 1) % NH0])
            cell1(h0r[T % NH0])

        if mode == "warm":
            dbg = const.tile([128, 128], F32, tag="dbg")
            nc.vector.tensor_copy(dbg[:, 0:64], h1[:])
            nc.vector.tensor_copy(dbg[:, 64:128], c1[:])
            nc.sync.dma_start(out_ap, dbg[:])
            return

        pred_head(0)

        nar = NP - 1
        artrip = nar // AR_BODY
        rem = nar - artrip * AR_BODY
        if artrip > 0:
            with tc.For_i(0, artrip, 1, hint_engines=hints) as av:
                for u in range(AR_BODY):
                    ar_step(av * AR_BODY + (u + 1))
        for u in range(rem):
            ar_step(artrip * AR_BODY + u + 1)

        nc.vector.tensor_scalar_add(preds[:], preds_ps[:], linbcol[:])
        nc.sync.dma_start(out_ap, preds[:])


# ---------------------------------------------------------------- entry


def make_in_maps(inputs, ncores=NCORES):
    shared = prep_shared(inputs)
    return [
        dict(shared, inpT=prep_core_input(inputs["input"], c)) for c in range(ncores)
    ]


_PROG_CACHE = {}


def kernel(**inputs):
    inp = np.asarray(inputs["input"], np.float32)
    assert inp.shape == (256, 2048), inp.shape
    NP = int(inputs["num_predictions"])
    if NP not in _PROG_CACHE:
        _PROG_CACHE[NP] = build_program(T_FULL, NP)
    nc = _PROG_CACHE[NP]
    in_maps = make_in_maps(inputs)
    res = bass_utils.run_bass_kernel_spmd(nc, in_maps, core_ids=list(range(NCORES)))
    return np.concatenate([r["out"] for r in res.results], axis=0)


if __name__ == "__main__":
    import reference

    inputs = {k: np.asarray(v) for k, v in reference.setup_inputs().items()}
    out = kernel(**inputs)
    exp = np.asarray(reference.reference(**reference.setup_inputs()))
    err = np.abs(out - exp).max()
    print("absmax err", err, "rel", err / np.abs(exp).max())


# revision 10
# speedup vs baseline: 1.3945x; 1.0114x over previous
"""Trainium2 Bass kernel for CnnLSTM (conv1x1 -> 2-layer LSTM -> AR decode).

Strategy: pure data parallel over batch (B=256 -> 32 per core x 8 cores).
Feature-major layout: gates live as [128 partitions = G-chunk, 32 free =
batch], hidden state as [128 part = h-dim chunk, 2*32].  Matmul operands are
fp16 (FWL weight loads, ~37ns per LDW+MM pair at N=32); accumulation/state
math is fp32.

Warmup pipelining: layer1 runs TWO steps behind layer0, so each tick carries
two independent dependency chains (cell0 of step s, cell1 of step s-2); the
xt/bias/ih0 projection for step s+1 is pre-issued one tick ahead, leaving
only the 16 hh0 matmuls plus one cell of ACT/DVE math on the recurrent
spine.  PSUM gate tiles are persistent even/odd ping-pongs so accumulation
groups can straddle tick boundaries.

AR decode: the linear head and the conv are fused into a single matmul
(x_{t+1} = relu((cw (x) lin_w)^T h1 + cb')), and the hh/bias matmuls of step
t+1 are pre-issued as soon as h0(t)/h1(t) land, so the serial spine is
relu -> ih0 -> cell0 -> ih1 -> cell1 -> (cwlin).

Biases are folded into the PSUM accumulation via a one-hot matmul (lhsT =
biasT [8,128], rhs = one-hot [8,256]).  Host side pre-transposes/permutes
weights into lhsT tile layouts, folds biases, and builds the relayouted
input; the device does all O(B*T*G*H) compute.
"""

import numpy as np

import concourse.bacc as bacc
import concourse.bass as bass
import concourse.mybir as mybir
import concourse.tile as tile
from concourse import bass_utils
from concourse.bass import ds

F16 = mybir.dt.float16
F32 = mybir.dt.float32
AF = mybir.ActivationFunctionType
ALU = mybir.AluOpType
ET = mybir.EngineType

# PSUM gate-slot order: i,i,f,f,o,o,g,g  (PyTorch gate order along G is i,f,g,o)
GPERM = [0, 1, 2, 3, 6, 7, 4, 5]
P = 128
B = 32  # batch per core
NCORES = 8
T_FULL = 2048
WARM_BODY = 64  # steady warmup ticks per For_i iteration
AR_BODY = 16  # AR steps per For_i iteration


# ---------------------------------------------------------------- host prep


def _relay_hh(W):
    # W [1024, 256] -> lhsT tiles [128, 2048] fp16; col block (k*8+s)*128+j
    # holds W.T[k*128+p, GPERM[s]*128+j]
    Wt = W.T.reshape(2, 128, 8, 128)
    Wt = Wt[:, :, GPERM, :]
    return np.ascontiguousarray(
        Wt.transpose(1, 0, 2, 3).reshape(128, 2048)
    ).astype(np.float16)


def _scale_g(W):
    # tanh(x) = 2*sigmoid(2x) - 1: pre-scale the g-gate rows (PyTorch gate
    # order i,f,g,o -> rows 512:768) by 2 so one sigmoid covers all gates.
    W = W.copy()
    W[512:768] *= 2.0
    return W


def prep_shared(inputs):
    f32 = np.float32
    g = lambda n: np.asarray(inputs[n], f32)
    W_ih0, W_hh0 = _scale_g(g("W_ih0")), _scale_g(g("W_hh0"))
    W_ih1, W_hh1 = _scale_g(g("W_ih1")), _scale_g(g("W_hh1"))
    b0 = _scale_g((g("b_ih0") + g("b_hh0"))[:, None])[:, 0]
    b1 = _scale_g((g("b_ih1") + g("b_hh1"))[:, None])[:, 0]
    conv_w, conv_b = g("conv_w"), g("conv_b")
    lin_w, lin_b = g("lin_w"), g("lin_b")

    # x-projection weights, slot-major: wih0u[p, s*128+j] = W_ih0.T[p, GPERM[s]*128+j]
    # (base_partition-64 row packing crashes the device, so keep all MMs at
    # partitions 0:64)
    Wt0 = W_ih0.T.reshape(64, 8, 128)[:, GPERM, :]
    wih0u = np.ascontiguousarray(Wt0.reshape(64, 1024)).astype(np.float16)

    def bias_lhsT(b):
        # biasT[s, p] = b[GPERM[s]*128 + p]  -> [8, 128] fp16
        br = b.reshape(8, 128)[GPERM]
        return np.ascontiguousarray(br).astype(np.float16)

    # one-hot rhs [8, 256]: onehot[s, g*32 + b] = (g == s)
    onehot = np.zeros((8, 256), np.float16)
    for s in range(8):
        onehot[s, s * 32 : (s + 1) * 32] = 1.0

    cw2 = np.tile(conv_w, 2)
    cb2 = np.tile(conv_b, 2)
    # fused conv(lin(h1)) weights: cps[c, b] = sum_h cw2[c]*lin_w[h]*h1[h, b]
    # lhsT block k: cwlinT[p, k*128 + c] = lin_w[k*128+p] * cw2[c]
    A = np.outer(lin_w[0], cw2)  # [256, 128]
    cwlinT = np.ascontiguousarray(
        np.concatenate([A[0:128], A[128:256]], axis=1)
    ).astype(np.float16)
    return {
        "whh0": _relay_hh(W_hh0),
        "wih1": _relay_hh(W_ih1),
        "whh1": _relay_hh(W_hh1),
        "wih0u": wih0u,
        "b0T": bias_lhsT(b0),
        "b1T": bias_lhsT(b1),
        "onehot": onehot,
        # interleaved [cw[c], cb[c]] pairs, replicated across partitions:
        # cwcb[p, 2c] = conv_w[c], cwcb[p, 2c+1] = conv_b[c]
        "cwcb": np.ascontiguousarray(
            np.broadcast_to(
                np.stack([conv_w, conv_b], axis=1).reshape(1, 128), (128, 128)
            )
        ).astype(f32),
        "linwT": np.ascontiguousarray(lin_w[0].reshape(2, 128).T).astype(np.float16),
        "cwlinT": cwlinT,
        "cb2col": (lin_b[0] * cw2 + cb2).astype(f32)[:, None],
        "linbcol": np.full((32, 1), lin_b[0], f32),
    }


def prep_core_input(input_full, core):
    # inpT[p, r*32+b] = input[32*core+b, 64*r + p%64], duplicated rows 64:128
    x = np.asarray(input_full, np.float32)[32 * core : 32 * core + 32]
    x = x.reshape(32, 32, 64)  # [b, r, k]
    one = x.transpose(2, 1, 0).reshape(64, 1024)  # [k, (r b)]
    return np.ascontiguousarray(np.concatenate([one, one], axis=0))


# ---------------------------------------------------------------- device IR


def build_program(T=T_FULL, NP=512, mode="full"):
    assert T >= 8 and T <= T_FULL and T % 4 == 0
    assert 2 <= NP <= 512
    nc = bacc.Bacc("TRN2", debug=False, enable_asserts=False, num_devices=NCORES)

    def din(name, shape, dt):
        return nc.dram_tensor(name, list(shape), dt, kind="ExternalInput").ap()

    t = {
        "whh0": din("whh0", (128, 2048), F16),
        "wih1": din("wih1", (128, 2048), F16),
        "whh1": din("whh1", (128, 2048), F16),
        "wih0u": din("wih0u", (64, 1024), F16),
        "inpT": din("inpT", (128, 1024), F32),
        "b0T": din("b0T", (8, 128), F16),
        "b1T": din("b1T", (8, 128), F16),
        "onehot": din("onehot", (8, 256), F16),
        "cwcb": din("cwcb", (128, 128), F32),
        "linwT": din("linwT", (128, 2), F16),
        "cwlinT": din("cwlinT", (128, 256), F16),
        "cb2col": din("cb2col", (128, 1), F32),
        "linbcol": din("linbcol", (32, 1), F32),
    }
    if mode in ("warm",):
        out_ap = nc.dram_tensor("out", [128, 128], F32, kind="ExternalOutput").ap()
    else:
        out_ap = nc.dram_tensor("out", [32, NP], F32, kind="ExternalOutput").ap()

    with tile.TileContext(nc) as tc:
        _emit(tc, nc, t, out_ap, T, NP, mode)
    nc.compile()
    return nc


def _emit(tc, nc, t, out_ap, T, NP, mode="full"):
    import contextlib

    with contextlib.ExitStack() as ctx:
        const = ctx.enter_context(tc.tile_pool(name="const", bufs=1))

        def load(name, shape, dt):
            tl = const.tile(list(shape), dt, tag=name)
            nc.sync.dma_start(tl[:], t[name])
            return tl

        whh0 = load("whh0", (128, 2048), F16)
        wih1 = load("wih1", (128, 2048), F16)
        whh1 = load("whh1", (128, 2048), F16)
        wih0u = load("wih0u", (64, 1024), F16)
        inpT = load("inpT", (128, 1024), F32)
        b0T = load("b0T", (8, 128), F16)
        b1T = load("b1T", (8, 128), F16)
        onehot = load("onehot", (8, 256), F16)
        cwcb = load("cwcb", (128, 128), F32)
        linwT = load("linwT", (128, 2), F16)
        cwlinT = load("cwlinT", (128, 256), F16)
        cb2col = load("cb2col", (128, 1), F32)
        linbcol = load("linbcol", (32, 1), F32)

        # persistent state.  h0 is a ring of 4 (cell1 lags cell0 by 2 steps in
        # warmup); h1/c0/c1 are single tiles.
        NH0 = 4
        h0r = []
        for i in range(NH0):
            h0i = const.tile([128, 64], F16, tag=f"h0r{i}", name=f"h0r{i}")
            h0r.append(h0i)
        c0 = const.tile([128, 64], F32, tag="c0")
        h1 = const.tile([128, 64], F16, tag="h1")
        c1 = const.tile([128, 64], F32, tag="c1")
        for st in (*h0r, c0, h1, c1):
            nc.vector.memset(st[:], 0.0)
        preds = const.tile([32, NP], F32, tag="preds")

        # persistent PSUM: even/odd gate tiles (accumulation groups straddle
        # tick boundaries), fused conv-lin tiles, preds accumulator.
        pconst = ctx.enter_context(tc.tile_pool(name="pconst", bufs=1, space="PSUM"))
        g0ab = []
        g1ab = []
        cpsab = []
        for i in range(2):
            g0i = pconst.tile([128, 256], F32, tag=f"g0{i}", name=f"g0{i}")
            g0ab.append(g0i)
            g1i = pconst.tile([128, 256], F32, tag=f"g1{i}", name=f"g1{i}")
            g1ab.append(g1i)
            cpsi = pconst.tile([128, 32], F32, tag=f"cps{i}", name=f"cps{i}")
            cpsab.append(cpsi)
        preds_ps = pconst.tile([32, NP], F32, tag="predps")

        spool = ctx.enter_context(tc.tile_pool(name="sg", bufs=3))
        tpool = ctx.enter_context(tc.tile_pool(name="tmp", bufs=3))
        xpool = ctx.enter_context(tc.tile_pool(name="xt", bufs=4))

        def bias_mm(g, bT, first):
            # g[:, s*32:(s+1)*32] += bias[GPERM[s]*128 + p]
            nc.tensor.matmul(
                g[:, 0:256], lhsT=bT[:], rhs=onehot[:], start=first, stop=False
            )

        def hh16(g, w, rhs_lo, rhs_hi, stop=False):
            for s in range(8):
                for k in range(2):
                    nc.tensor.matmul(
                        g[:, s * 32 : (s + 1) * 32],
                        lhsT=w[:, (k * 8 + s) * 128 : (k * 8 + s + 1) * 128],
                        rhs=rhs_lo if k == 0 else rhs_hi,
                        start=False,
                        stop=stop and (s == 7 and k == 1),
                    )

        def ih0_8(g, xt, stop):
            for s in range(8):
                nc.tensor.matmul(
                    g[:, s * 32 : (s + 1) * 32],
                    lhsT=wih0u[:, s * 128 : (s + 1) * 128],
                    rhs=xt[0:64, :],
                    start=False,
                    stop=stop and s == 7,
                )

        def cell_math(g, tagp, h_dst, c_st):
            # g: [128, 256] PSUM (slots i,i,f,f,o,o,2g,2g).  One sigmoid over
            # all 256 cols; tanh(g) is recovered as 2*sigmoid(2g)-1.
            sg = spool.tile([128, 256], F32, tag=tagp + "s")
            nc.scalar.activation(sg[:], g[:, 0:256], AF.Sigmoid)
            m1 = tpool.tile([128, 64], F32, tag=tagp + "m1")
            nc.vector.tensor_mul(m1[:], sg[:, 64:128], c_st[:])
            gt = tpool.tile([128, 64], F32, tag=tagp + "g")
            nc.vector.tensor_scalar(gt[:], sg[:, 192:256], 2.0, -1.0,
                                    ALU.mult, ALU.add)
            m2 = tpool.tile([128, 64], F32, tag=tagp + "m2")
            nc.vector.tensor_mul(m2[:], sg[:, 0:64], gt[:])
            nc.vector.tensor_add(c_st[:], m1[:], m2[:])
            tcc = tpool.tile([128, 64], F32, tag=tagp + "t")
            nc.scalar.activation(tcc[:], c_st[:], AF.Tanh)
            nc.vector.tensor_mul(h_dst[:], sg[:, 128:192], tcc[:])

        def make_xt(cur_or_static, r):
            # xt = relu(cw * u_r + cb); cur_or_static is a [128,2] snapshot
            # tile (loop body) or a static cwcb column pair (unrolled code).
            xt = xpool.tile([128, 32], F16, tag="xt")
            nc.scalar.activation(
                xt[:],
                inpT[:, r * 32 : (r + 1) * 32],
                AF.Relu,
                bias=cur_or_static[:, 1:2],
                scale=cur_or_static[:, 0:1],
            )
            return xt

        def pre0_warm(s, cur):
            # pre-issue for warm step s: xt(s) + bias0 + ih0 into g0[s%2].
            # x for (1-based) step s is reference step t = s-1.
            xt = make_xt(cur, (s - 1) % 32)
            g = g0ab[s % 2]
            bias_mm(g, b0T, True)
            ih0_8(g, xt, stop=False)

        def fin0(s):
            # finish warm cell0 of step s: hh0 + cell math
            g = g0ab[s % 2]
            hh16(g, whh0, h0r[(s - 1) % NH0][:, 0:32], h0r[(s - 1) % NH0][:, 32:64],
                 stop=True)
            cell_math(g, "l0", h0r[s % NH0], c0)

        def cell1_full(s, h0_in):
            # full cell1 (bias+ih1+hh1+math) for (1-based) step s
            g = g1ab[s % 2]
            bias_mm(g, b1T, True)
            hh16(g, wih1, h0_in[:, 0:32], h0_in[:, 32:64])
            hh16(g, whh1, h1[:, 0:32], h1[:, 32:64], stop=True)
            cell_math(g, "l1", h1, c1)

        def warm_tick(s, cur_next):
            # steady tick: pre-issue step s+1 (xt/bias/ih0 — always ready, fills
            # engine FIFOs), finish cell0 of step s (its h0(s-1) dependency
            # lands first), then cell1 of step s-2 (its h1(s-3) lands later).
            if s < T:
                pre0_warm(s + 1, cur_next)
            fin0(s)
            cell1_full(s - 2, h0r[(s - 2) % NH0])

        def pred_col(col):
            for k in range(2):
                nc.tensor.matmul(
                    preds_ps[:, ds(col, 1)],
                    lhsT=h1[:, k * 32 : (k + 1) * 32],
                    rhs=linwT[:, k : k + 1],
                    start=(k == 0),
                    stop=(k == 1),
                )

        def cwlin(par):
            # cps[par] = (cw (x) lin_w)^T @ h1   [fused conv(lin(h1))]
            for k in range(2):
                nc.tensor.matmul(
                    cpsab[par][:],
                    lhsT=cwlinT[:, k * 128 : (k + 1) * 128],
                    rhs=h1[:, k * 32 : (k + 1) * 32],
                    start=(k == 0),
                    stop=(k == 1),
                )

        def ar_preissue(u):
            # after h0/h1 of step u-1 are final: start gate groups for step u
            g0 = g0ab[u % 2]
            bias_mm(g0, b0T, True)
            hh16(g0, whh0, h0r[0][:, 0:32], h0r[0][:, 32:64])
            cwlin(u % 2)
            g1 = g1ab[u % 2]
            bias_mm(g1, b1T, True)
            hh16(g1, whh1, h1[:, 0:32], h1[:, 32:64])

        def ar_step(u):
            # finish step u (cps/g0-partials/g1-partials pre-issued), then
            # pre-issue step u+1 interleaved at the right spots.
            par = u % 2
            xt = xpool.tile([128, 32], F16, tag="xt")
            nc.scalar.activation(xt[:], cpsab[par][:], AF.Relu, bias=cb2col[:])
            g0 = g0ab[par]
            ih0_8(g0, xt, stop=True)
            cell_math(g0, "l0", h0r[0], c0)
            g1 = g1ab[par]
            hh16(g1, wih1, h0r[0][:, 0:32], h0r[0][:, 32:64], stop=True)
            # off-spine: start next step's g0 group (needs h0(u) only)
            np_ = (u + 1) % 2
            g0n = g0ab[np_]
            bias_mm(g0n, b0T, True)
            hh16(g0n, whh0, h0r[0][:, 0:32], h0r[0][:, 32:64])
            cell_math(g1, "l1", h1, c1)
            # spine: fused conv-lin for step u+1 first, then off-spine rest
            cwlin(np_)
            g1n = g1ab[np_]
            bias_mm(g1n, b1T, True)
            hh16(g1n, whh1, h1[:, 0:32], h1[:, 32:64])
            pred_col(u)

        hints = (ET.PE, ET.DVE, ET.Activation)

        # ---------------- warmup scan (ticks with layer1 lagging by 2) ----
        if mode != "ar":
            # prologue: pre-issue step 1; finish steps 1,2 (no cell1 yet)
            pre0_warm(1, cwcb[:, 0:2])
            pre0_warm(2, cwcb[:, 0:2])
            fin0(1)
            pre0_warm(3, cwcb[:, 0:2])
            fin0(2)

            # steady ticks: s = 3 .. T  (cell0 step s, cell1 step s-2)
            nsteady = T - 2
            ntrip = max(0, (nsteady - 2) // WARM_BODY)
            loop_end = 3 + ntrip * WARM_BODY  # first static-tail step
            if ntrip > 0:
                with tc.For_i(0, ntrip, 1, hint_engines=hints) as iv:
                    # ACT scale/bias operands do not support register offsets
                    # on HW, so snapshot this body's three (cw, cb) pairs into
                    # static tiles via DMA (register offsets on DMA are fine).
                    curs = []
                    for third in range(3):
                        cur = xpool.tile([128, 2], F32, tag=f"cwcur{third}")
                        nc.sync.dma_start(cur[:], cwcb[:, ds(iv * 4 + third * 2, 2)])
                        curs.append(cur)
                    for j in range(WARM_BODY):
                        # actual step s = 3 + iv*WARM_BODY + j; pre-issue is
                        # for step s+1 whose x index is s; s//32 = 2*iv +
                        # (3+j)//32.
                        warm_tick(3 + j, curs[(3 + j) // 32])
            # static tail: steps loop_end .. T
            for s in range(loop_end, T + 1):
                p_ = min(s // 32, 63)
                warm_tick(s, cwcb[:, 2 * p_ : 2 * p_ + 2])
            # epilogue: cell1 steps T-1, T
            cell1_full(T - 1, h0r[(T - 1) % NH0])
            cell1_full(T, h0r[T % NH0])

        if mode == "warm":
            dbg = const.tile([128, 128], F32, tag="dbg")
            nc.vector.tensor_copy(dbg[:, 0:64], h1[:])
            nc.vector.tensor_copy(dbg[:, 64:128], c1[:])
            nc.sync.dma_start(out_ap, dbg[:])
            return

        pred_col(0)
        ar_preissue(1)

        nar = NP - 1
        artrip = nar // AR_BODY
        rem = nar - artrip * AR_BODY
        if artrip > 0:
            with tc.For_i(0, artrip, 1, hint_engines=hints) as av:
                for u in range(AR_BODY):
                    ar_step(av * AR_BODY + (u + 1))
        for u in range(rem):
            ar_step(artrip * AR_BODY + u + 1)

        nc.vector.tensor_scalar_add(preds[:], preds_ps[:], linbcol[:])
        nc.sync.dma_start(out_ap, preds[:])


# ---------------------------------------------------------------- entry


def make_in_maps(inputs, ncores=NCORES):
    shared = prep_shared(inputs)
    return [
        dict(shared, inpT=prep_core_input(inputs["input"], c)) for c in range(ncores)
    ]


_PROG_CACHE = {}


def kernel(**inputs):
    inp = np.asarray(inputs["input"], np.float32)
    assert inp.shape == (256, 2048), inp.shape
    NP = int(inputs["num_predictions"])
    if NP not in _PROG_CACHE:
        _PROG_CACHE[NP] = build_program(T_FULL, NP)
    nc = _PROG_CACHE[NP]
    in_maps = make_in_maps(inputs)
    res = bass_utils.run_bass_kernel_spmd(nc, in_maps, core_ids=list(range(NCORES)))
    return np.concatenate([r["out"] for r in res.results], axis=0)


if __name__ == "__main__":
    import reference

    inputs = {k: np.asarray(v) for k, v in reference.setup_inputs().items()}
    out = kernel(**inputs)
    exp = np.asarray(reference.reference(**reference.setup_inputs()))
    err = np.abs(out - exp).max()
    print("absmax err", err, "rel", err / np.abs(exp).max())
